# revision 1
# baseline (speedup 1.0000x reference)
"""GNN message-passing aggregator on 8 Trainium2 NeuronCores.

  h = relu(relu(z @ U1 + c1) @ U2 + c2)
  z = segment_sum(relu(relu(y[src] @ W1 + b1) @ W2 + b2), dst)

Strategy:
  * MLP(y[src]) == MLP(y)[src]: compute the pre-MLP once per node (m), then the
    per-edge work collapses to gather m[src] + segment-sum by dst.
  * Edges are sharded by dst ownership (6250 nodes per core) on the host, so the
    segment-sum is core-local: no collectives at all.
  * Per core the dst nodes form 49 windows of 128; a window's edges are packed
    into 128-slot columns.  Each column is gathered from the m table in HBM via
    dma_gather (int16 indices -> the table is addressed as lo/hi halves) and
    accumulated into the window's PSUM tile with a matmul whose stationary
    operand is a selection matrix P[p, n] = (nodeid[p] == n), built on the DVE
    from a host-streamed compact nodeid vector.  PSUM accumulation gives an
    exact fp32 segment sum.  The gather (GPSIMD Q7 descriptor generation,
    ~8.4 ns/slot) is the hard floor of this kernel; everything else overlaps it.
  * Phase 1 (m) runs before the gathers that need it: lo-half gathers only wait
    on the lo half of the m table (junction NOP deps), hi on the rest.
  * Phase 3 (update MLP) is tiled in groups of 4 windows so it pipelines into
    the tail of the gather stream.
"""

import os

import numpy as np

# ---------------------------------------------------------------- constants
N_NODES = 50000
D = 64
NC = 8                      # cores
W = 128                     # window size == psum partitions
SPLIT = 32768               # int16-addressable half of the m table
OP_COLS = 8                 # gather columns per dma_gather op (8*128=1024 idx)
CHUNK = 512                 # dense-MLP T-major matmul chunk
GRP = 4                     # windows per phase-3 group (GRP*W == CHUNK)

_COMPILED = {}


def _dims():
    npc = N_NODES // NC
    nw = (npc + W - 1) // W
    npad = nw * W
    mt_rows = ((N_NODES + 127) // 128) * 128
    return npc, nw, npad, mt_rows


# ------------------------------------------------------------ host schedule
def _host_schedule(src, dst):
    """Shard edges by dst ownership, balance nodes into windows, pack columns.

    Returns (sched, per_core): sched is the shared compile-time schedule
    (identical across cores); per_core holds the input arrays per core.
    """
    NPC, NW, NPAD, _ = _dims()

    percore_groups = []
    percore_perm = []
    clo = np.zeros((NC, NW), np.int64)
    chi = np.zeros((NC, NW), np.int64)

    for c in range(NC):
        lo_n, hi_n = c * NPC, (c + 1) * NPC
        sel = (dst >= lo_n) & (dst < hi_n)
        s = src[sel].astype(np.int64)
        d = (dst[sel] - lo_n).astype(np.int64)
        deg = np.bincount(d, minlength=NPC)

        # balance nodes into NW windows by degree (greedy, descending)
        order = np.argsort(-deg, kind="stable")
        wload = np.zeros(NW, np.int64)
        wcount = np.zeros(NW, np.int64)
        assign = np.zeros(NPC, np.int64)
        label = np.zeros(NPC, np.int64)
        for n in order:
            wavail = np.flatnonzero(wcount < W)
            wsel = wavail[np.argmin(wload[wavail])]
            assign[n] = wsel
            label[n] = wcount[wsel]
            wcount[wsel] += 1
            wload[wsel] += deg[n]

        ew = assign[d]
        is_lo = s < SPLIT
        groups = {}
        for wdw in range(NW):
            m_w = ew == wdw
            for t in range(2):
                m_t = m_w & (is_lo if t == 0 else ~is_lo)
                es = s[m_t]
                el = label[d[m_t]]
                o = np.argsort(es, kind="stable")   # src-sorted for HBM locality
                groups[(wdw, t)] = (es[o], el[o])
                cnt = (len(es) + W - 1) // W
                if t == 0:
                    clo[c, wdw] = cnt
                else:
                    chi[c, wdw] = cnt
        percore_groups.append(groups)
        perm = np.full(NPAD, -1, np.int64)
        perm[assign * W + label] = np.arange(NPC) + lo_n
        percore_perm.append(perm)

    CLo = np.maximum(clo.max(0), 1)
    CHi = np.maximum(chi.max(0), 1)
    lo_off = np.concatenate([[0], np.cumsum(CLo)])
    hi_off = np.concatenate([[0], np.cumsum(CHi)])
    n_lo, n_hi = int(lo_off[-1]), int(hi_off[-1])

    per_core = []
    for c in range(NC):
        groups = percore_groups[c]
        idx = [np.zeros(n_lo * W, np.int16), np.zeros(n_hi * W, np.int16)]
        nid = [np.full(n_lo * W, -1.0, np.float32),
               np.full(n_hi * W, -1.0, np.float32)]
        for wdw in range(NW):
            for t, (cnt, off) in enumerate(((CLo, lo_off), (CHi, hi_off))):
                es, el = groups[(wdw, t)]
                base = int(off[wdw]) * W
                k = len(es)
                idx[t][base:base + k] = (es if t == 0 else es - SPLIT).astype(np.int16)
                nid[t][base:base + k] = el.astype(np.float32)
        # wrap idx into per-op [16, ni/16] layout replicated to 128 partitions
        wrapped = []
        for t, ncols in ((0, n_lo), (1, n_hi)):
            flat = idx[t]
            pos, blocks = 0, []
            while pos < ncols:
                k = min(OP_COLS, ncols - pos)
                ni = k * W
                op = flat[pos * W:(pos + k) * W]
                blk = op.reshape(ni // 16, 16).T            # [16, ni/16]
                blocks.append(np.tile(blk, (8, 1)))          # [128, ni/16]
                pos += k
            wrapped.append(np.concatenate(blocks, axis=1) if blocks
                           else np.zeros((128, 8), np.int16))
        nodeid_2d = np.concatenate([nid[0], nid[1]]).reshape(n_lo + n_hi, W).T
        per_core.append({
            "idx_lo": np.ascontiguousarray(wrapped[0]),
            "idx_hi": np.ascontiguousarray(wrapped[1]),
            "nodeid": np.ascontiguousarray(nodeid_2d.astype(np.float32)),
            "perm": percore_perm[c],
        })

    sched = {"CLo": CLo.astype(int).tolist(), "CHi": CHi.astype(int).tolist(),
             "n_lo": n_lo, "n_hi": n_hi}
    return sched, per_core


# ------------------------------------------------------------- bass program
def _build_program(sched):
    import concourse.bacc as bacc
    import concourse.mybir as mybir
    import concourse.tile as tile
    from concourse.masks import make_identity
    from concourse.tile import add_dep_helper

    f32 = mybir.dt.float32
    i16 = mybir.dt.int16
    Relu = mybir.ActivationFunctionType.Relu

    NPC, NW, NPAD, MT_ROWS = _dims()
    CLo, CHi = sched["CLo"], sched["CHi"]
    n_lo, n_hi = sched["n_lo"], sched["n_hi"]
    n_cols = n_lo + n_hi
    NCH = (MT_ROWS + CHUNK - 1) // CHUNK
    LO_CH = SPLIT // CHUNK          # chunks 0..LO_CH-1 cover m rows < SPLIT

    nc = bacc.Bacc()
    yT_in = nc.dram_tensor("yT", [D + 1, MT_ROWS], f32, kind="ExternalInput")
    wb1_in = nc.dram_tensor("wb1", [D + 1, D], f32, kind="ExternalInput")
    wb2_in = nc.dram_tensor("wb2", [D + 1, D], f32, kind="ExternalInput")
    ub1_in = nc.dram_tensor("ub1", [D + 1, D], f32, kind="ExternalInput")
    ub2_in = nc.dram_tensor("ub2", [D + 1, D], f32, kind="ExternalInput")
    idxlo_in = nc.dram_tensor("idx_lo", [128, n_lo * 8], i16, kind="ExternalInput")
    idxhi_in = nc.dram_tensor("idx_hi", [128, n_hi * 8], i16, kind="ExternalInput")
    nodeid_in = nc.dram_tensor("nodeid", [128, n_cols], f32, kind="ExternalInput")
    iota_in = nc.dram_tensor("iota128", [128, 128], f32, kind="ExternalInput")
    m_dram = nc.dram_tensor("m_scratch", [MT_ROWS, D], f32, kind="Internal")
    h_out = nc.dram_tensor("h_out", [NPAD, D], f32, kind="ExternalOutput")
    debug = bool(int(os.environ.get("KERNEL_DEBUG_Z", "0")))
    if debug:
        z_out = nc.dram_tensor("z_out", [NPAD, D], f32, kind="ExternalOutput")
        m_out = nc.dram_tensor("m_out", [MT_ROWS, D], f32, kind="ExternalOutput")

    with tile.TileContext(nc) as tc:
        with tc.tile_pool(name="const", bufs=1) as cpool, \
             tc.tile_pool(name="idxp", bufs=1) as idxp, \
             tc.tile_pool(name="zpool", bufs=1) as zpool:
            wb1 = cpool.tile([D + 1, D], f32, tag="wb1")
            wb2 = cpool.tile([D + 1, D], f32, tag="wb2")
            ub1 = cpool.tile([D + 1, D], f32, tag="ub1")
            ub2 = cpool.tile([D + 1, D], f32, tag="ub2")
            iota = cpool.tile([128, 128], f32, tag="iota")
            ident = cpool.tile([128, 128], f32, tag="ident")
            nc.sync.dma_start(out=wb1[:], in_=wb1_in[:])
            nc.sync.dma_start(out=wb2[:], in_=wb2_in[:])
            nc.sync.dma_start(out=ub1[:], in_=ub1_in[:])
            nc.sync.dma_start(out=ub2[:], in_=ub2_in[:])
            nc.sync.dma_start(out=iota[:], in_=iota_in[:])
            make_identity(nc, ident[:])

            # phase-2 static inputs: load up front (independent of phase 1)
            idx_lo_t = idxp.tile([128, n_lo * 8], i16, tag="ilo")
            idx_hi_t = idxp.tile([128, n_hi * 8], i16, tag="ihi")
            idx_t = [idx_lo_t, idx_hi_t]
            nc.sync.dma_start(out=idx_lo_t[:], in_=idxlo_in[:])
            nc.sync.dma_start(out=idx_hi_t[:], in_=idxhi_in[:])
            nodeid_t = idxp.tile([128, n_cols], f32, tag="nid")
            nc.sync.dma_start(out=nodeid_t[:], in_=nodeid_in[:])

            # ------------ phase 1: m = relu(relu(y@W1+b1)@W2+b2) -> m_dram ---
            m_writes = []        # per-chunk m-write DMA instructions
            with tc.tile_pool(name="p1y", bufs=3) as p1y, \
                 tc.tile_pool(name="p1h", bufs=1) as p1h, \
                 tc.tile_pool(name="p1m", bufs=3) as p1m, \
                 tc.tile_pool(name="p1ps", bufs=2, space="PSUM") as p1ps, \
                 tc.tile_pool(name="p1ps2", bufs=3, space="PSUM") as p1ps2:
                # two persistent h1 buffers with the ones-row preset ONCE, so
                # the per-chunk ACT(relu) does not serialize against a memset.
                h1a = p1h.tile([D + 1, CHUNK], f32, tag="h1a")
                h1b = p1h.tile([D + 1, CHUNK], f32, tag="h1b")
                nc.gpsimd.memset(h1a[D:D + 1, :], 1.0)
                nc.gpsimd.memset(h1b[D:D + 1, :], 1.0)
                h1bufs = [h1a, h1b]
                for ch in range(NCH):
                    c0 = ch * CHUNK
                    cw = min(CHUNK, MT_ROWS - c0)
                    ytile = p1y.tile([D + 1, CHUNK], f32, tag="ytile")
                    nc.sync.dma_start(out=ytile[:, :cw], in_=yT_in[:, c0:c0 + cw])
                    ps = p1ps.tile([D, CHUNK], f32, tag="ps1")
                    nc.tensor.matmul(out=ps[:, :cw], lhsT=wb1[:], rhs=ytile[:, :cw],
                                     start=True, stop=True)
                    h1c = h1bufs[ch % 2]
                    nc.scalar.activation(out=h1c[:D, :cw], in_=ps[:, :cw], func=Relu)
                    mch = p1m.tile([128, (CHUNK // 128) * D], f32, tag="mch")
                    for i in range(cw // 128):
                        ps2 = p1ps2.tile([128, D], f32, tag="ps2")
                        nc.tensor.matmul(out=ps2[:],
                                         lhsT=h1c[:, i * 128:(i + 1) * 128],
                                         rhs=wb2[:], start=True, stop=True)
                        nc.vector.tensor_scalar_max(
                            out=mch[:, i * D:(i + 1) * D], in0=ps2[:], scalar1=0.0)
                    wri = nc.scalar.dma_start(
                        out=m_dram[c0:c0 + cw, :].rearrange(
                            "(t p) d -> p t d", p=128),
                        in_=mch[:, :(cw // 128) * D].rearrange(
                            "p (t d) -> p t d", d=D))
                    m_writes.append(wri)

            # junction NOPs: gathers of each table half wait only on the
            # m-writes covering that half (Tile does not track DRAM RAW deps).
            jlo = nc.sync.nop(nofuse=True)
            jhi = nc.sync.nop(nofuse=True)
            for ch, wri in enumerate(m_writes):
                tgt = jlo if ch < LO_CH else jhi
                add_dep_helper(tgt.ins, wri.ins, sync=True,
                               reason="m table half complete")
            # hi junction also needs... (hi gathers only touch rows >= SPLIT)
            junction = [jlo, jhi]

            # ------------ phase 2: gather + segment-sum ----------------------
            z_grps = []
            NGRP = (NW + GRP - 1) // GRP
            with tc.tile_pool(name="gpool", bufs=6) as gpool, \
                 tc.tile_pool(name="ppool", bufs=4) as ppool, \
                 tc.tile_pool(name="zg", bufs=NGRP) as zgp, \
                 tc.tile_pool(name="wps", bufs=4, space="PSUM") as wps:
                tables = [m_dram[0:SPLIT, :], m_dram[SPLIT:MT_ROWS, :]]
                ncols_t = [n_lo, n_hi]
                g_tiles = [{}, {}]

                def ensure_op(t, col):
                    o = col // OP_COLS
                    if o in g_tiles[t]:
                        return g_tiles[t][o]
                    k = min(OP_COLS, ncols_t[t] - o * OP_COLS)
                    g = gpool.tile([128, k, D], f32, tag=f"g{t}")
                    ni = k * W
                    gi = nc.gpsimd.dma_gather(
                        out_ap=g[:], in_ap=tables[t],
                        idxs_ap=idx_t[t][:, o * OP_COLS * 8:o * OP_COLS * 8 + k * 8],
                        num_idxs=ni, num_idxs_reg=ni, elem_size=D)
                    add_dep_helper(gi.ins, junction[t].ins, sync=True,
                                   reason="gather after m half ready")
                    g_tiles[t][o] = g
                    return g

                lo_base, hi_base = 0, 0
                zgrp = None
                for wdw in range(NW):
                    if wdw % GRP == 0:
                        gw = min(GRP, NW - wdw)
                        zgrp = zgp.tile([128, gw * D], f32, tag="zgt")
                        z_grps.append(zgrp)
                    zw = wps.tile([128, D], f32, tag="zw")
                    total = CLo[wdw] + CHi[wdw]
                    ci = 0
                    for t, cnt, base in ((0, CLo[wdw], lo_base),
                                         (1, CHi[wdw], hi_base)):
                        for j in range(cnt):
                            col = base + j
                            g = ensure_op(t, col)
                            sub = col - (col // OP_COLS) * OP_COLS
                            gcol = col if t == 0 else n_lo + col
                            P = ppool.tile([128, 128], f32, tag="P")
                            nc.vector.tensor_tensor(
                                out=P[:],
                                in0=nodeid_t[:, gcol:gcol + 1].to_broadcast(
                                    [128, 128]),
                                in1=iota[:], op=mybir.AluOpType.is_equal)
                            nc.tensor.matmul(out=zw[:], lhsT=P[:],
                                             rhs=g[:, sub, :],
                                             start=(ci == 0),
                                             stop=(ci == total - 1))
                            ci += 1
                    lo_base += CLo[wdw]
                    hi_base += CHi[wdw]
                    nc.scalar.copy(out=zgrp[:, (wdw % GRP) * D:(wdw % GRP + 1) * D],
                                   in_=zw[:])

                if debug:
                    for gi_, zgrp_ in enumerate(z_grps):
                        gw = zgrp_.shape[1] // D
                        nc.sync.dma_start(
                            out=z_out[gi_ * GRP * 128:(gi_ * GRP + gw) * 128, :]
                                .rearrange("(t p) d -> p t d", p=128),
                            in_=zgrp_[:].rearrange("p (t d) -> p t d", d=D))
                    mo = nc.sync.dma_start(out=m_out[:], in_=m_dram[:])
                    add_dep_helper(mo.ins, junction[0].ins, sync=True, reason="dbg")
                    add_dep_helper(mo.ins, junction[1].ins, sync=True, reason="dbg")

                # ------------ phase 3: h = relu(relu(z@U1+c1)@U2+c2) ---------
                # grouped by GRP windows so it pipelines into the gather tail
                with tc.tile_pool(name="p3z", bufs=3) as p3z, \
                     tc.tile_pool(name="p3g", bufs=3) as p3g, \
                     tc.tile_pool(name="p3h", bufs=3) as p3h, \
                     tc.tile_pool(name="p3ps", bufs=1, space="PSUM") as p3ps, \
                     tc.tile_pool(name="p3psb", bufs=2, space="PSUM") as p3psb, \
                     tc.tile_pool(name="p3ps2", bufs=1, space="PSUM") as p3ps2:
                    for gi_ in range(NGRP):
                        zgrp = z_grps[gi_]
                        gw = zgrp.shape[1] // D
                        cw = gw * 128
                        zTg = p3z.tile([D + 1, GRP * 128], f32, tag="zTg")
                        nc.gpsimd.memset(zTg[D:D + 1, :cw], 1.0)
                        for k in range(gw):
                            pst = p3ps.tile([D, 128], f32, tag="pst")
                            nc.tensor.transpose(out=pst[:],
                                                in_=zgrp[:, k * D:(k + 1) * D],
                                                identity=ident[:])
                            nc.vector.tensor_copy(
                                out=zTg[:D, k * 128:(k + 1) * 128], in_=pst[:])
                        ps = p3psb.tile([D, GRP * 128], f32, tag="ps3")
                        nc.tensor.matmul(out=ps[:, :cw], lhsT=ub1[:],
                                         rhs=zTg[:, :cw], start=True, stop=True)
                        g1Tg = p3g.tile([D + 1, GRP * 128], f32, tag="g1Tg")
                        nc.scalar.activation(out=g1Tg[:D, :cw], in_=ps[:, :cw],
                                             func=Relu)
                        nc.gpsimd.memset(g1Tg[D:D + 1, :cw], 1.0)
                        h_sb = p3h.tile([128, GRP * D], f32, tag="h_sb")
                        for k in range(gw):
                            ps2 = p3ps2.tile([128, D], f32, tag="ps4")
                            nc.tensor.matmul(out=ps2[:],
                                             lhsT=g1Tg[:, k * 128:(k + 1) * 128],
                                             rhs=ub2[:], start=True, stop=True)
                            nc.vector.tensor_scalar_max(
                                out=h_sb[:, k * D:(k + 1) * D], in0=ps2[:],
                                scalar1=0.0)
                        nc.sync.dma_start(
                            out=h_out[gi_ * GRP * 128:(gi_ * GRP + gw) * 128, :]
                                .rearrange("(t p) d -> p t d", p=128),
                            in_=h_sb[:, :gw * D].rearrange("p (t d) -> p t d", d=D))

    nc.compile()
    return nc


# ------------------------------------------------------------------- kernel
def kernel(**inputs):
    from concourse.bass_utils import run_bass_kernel_spmd

    NPC, NW, NPAD, MT_ROWS = _dims()
    y = np.asarray(inputs["y"], np.float32)
    src = np.asarray(inputs["src"])
    dst = np.asarray(inputs["dst"])
    Ws = {k: np.asarray(inputs[k], np.float32)
          for k in ("W1", "b1", "W2", "b2", "U1", "c1", "U2", "c2")}

    sched, per_core = _host_schedule(src, dst)
    key = (tuple(sched["CLo"]), tuple(sched["CHi"]))
    if key not in _COMPILED:
        _COMPILED[key] = _build_program(sched)
    nc = _COMPILED[key]

    yT = np.zeros((D + 1, MT_ROWS), np.float32)
    yT[:D, :N_NODES] = y.T
    yT[D, :] = 1.0
    wb1 = np.concatenate([Ws["W1"], Ws["b1"][None, :]], axis=0)
    wb2 = np.concatenate([Ws["W2"], Ws["b2"][None, :]], axis=0)
    ub1 = np.concatenate([Ws["U1"], Ws["c1"][None, :]], axis=0)
    ub2 = np.concatenate([Ws["U2"], Ws["c2"][None, :]], axis=0)
    iota = np.tile(np.arange(128, dtype=np.float32), (128, 1))

    in_maps = []
    for c in range(NC):
        pc = per_core[c]
        in_maps.append({
            "yT": yT, "wb1": wb1, "wb2": wb2, "ub1": ub1, "ub2": ub2,
            "idx_lo": pc["idx_lo"], "idx_hi": pc["idx_hi"],
            "nodeid": pc["nodeid"], "iota128": iota,
        })

    res = run_bass_kernel_spmd(nc, in_maps, core_ids=list(range(NC)),
                               trace=bool(int(os.environ.get("KERNEL_TRACE", "0"))))
    kernel.last_results = res
    kernel.last_exec_time_ns = res.exec_time_ns

    h_full = np.zeros((N_NODES, D), np.float32)
    for c in range(NC):
        out = res.results[c]["h_out"]
        perm = per_core[c]["perm"]
        valid = perm >= 0
        h_full[perm[valid]] = out[valid]
    return h_full



# revision 9
# speedup vs baseline: 1.7843x; 1.7843x over previous
"""GNN message-passing aggregator on 8 Trainium2 NeuronCores.

  h = relu(relu(z @ U1 + c1) @ U2 + c2)
  z = segment_sum(relu(relu(y[src] @ W1 + b1) @ W2 + b2), dst)

Strategy (v2):
  * MLP(y[src]) == MLP(y)[src]: compute the pre-MLP once per node (m table in
    HBM), then per-edge work collapses to gather m[src] + segment-sum by dst.
  * Edges are sharded by dst ownership (6250 nodes per core) on the host, so
    the segment-sum is core-local: no collectives.
  * Desc-gen parallelism: dma_gather ops round-robin over 4 SWDGE queues, so
    descriptor generation runs on all 8 GPSIMD Q7 cores (4 tx/rx pairs)
    instead of 2 -> ~4x faster gather stream than a single queue.
  * Identity-packed columns: window w's column c holds, at slot p, the c-th
    edge of label p (slot == target partition).  The segment-sum for these
    columns is a plain DVE add of the gathered tile into the window
    accumulator -- no P matrix, no PE matmul.  Labels with fewer than c edges
    point the slot at a dedicated zero row of the m table.  Edges beyond the
    per-label cap go into compact columns handled by the classic
    selection-matrix matmul into PSUM.
  * Phase 1 runs in bf16 (4x PE throughput); a per-chunk junction chain lets
    each gather op start as soon as the m-table rows it actually reads are
    written, overlapping phase 1 with the gather stream.
  * The m table is split in int16-addressable halves (rows 0..32767 /
    32768..), each with its zero row first so hole slots depend only on the
    half's first chunk.
"""

import os

import numpy as np

# ---------------------------------------------------------------- constants
N_NODES = 50000
D = 64
NC = 8                      # cores
W = 128                     # window size == partitions
NPC = N_NODES // NC         # 6250 dst nodes per core
NW = (NPC + W - 1) // W     # 49 windows
NPAD = NW * W               # 6272
SPLIT = 32768               # rows in the lo half of the m table
LO_NODES = SPLIT - 1        # node ids 0..32766 live in the lo half
MT_ROWS = 50176             # 98 chunks of 512 (>= 2 + N_NODES rounded up)
CHUNK = 512                 # phase-1 chunk (rows per m-write)
NCH = MT_ROWS // CHUNK      # 98
LO_CH = SPLIT // CHUNK      # 64 chunks cover the lo half
OP_COLS = 8                 # gather columns per dma_gather op (8*128 idx)
GRP = 4                     # windows per phase-3 group
NQ = 4                      # SWDGE queues

_COMPILED = {}


# ------------------------------------------------------------ host schedule
def _host_schedule(src, dst):
    """Shard edges by dst ownership; build identity-packed + compact columns.

    m-table row map: row 0 = zero row (lo), rows 1..32767 = nodes 0..32766,
    row 32768 = zero row (hi), rows 32769.. = nodes 32767..49999.
    idx_lo(n) = n + 1;  idx_hi(n) = n - LO_NODES + 1  (both >= 1; 0 is never
    used as a real lo index, hole slots use idx 0 which reads the zero row of
    the hi half / for the lo half holes use idx 0 too -> row 0 = zero).
    """
    src = np.asarray(src, np.int64)
    dst = np.asarray(dst, np.int64)

    per_core_raw = []
    # per-core, per-window leftover column counts (for shared maxima)
    comp_cnt = {0: np.zeros((NC, NW), np.int64), 1: np.zeros((NC, NW), np.int64)}
    cap_candidates = None

    # ---- pass 1: per-core edge grouping + degree-balanced window assign ----
    for c in range(NC):
        lo_n, hi_n = c * NPC, (c + 1) * NPC
        sel = (dst >= lo_n) & (dst < hi_n)
        s = src[sel]
        d = dst[sel] - lo_n
        deg = np.bincount(d, minlength=NPC)

        order = np.argsort(-deg, kind="stable")
        wload = np.zeros(NW, np.int64)
        wcount = np.zeros(NW, np.int64)
        assign = np.zeros(NPC, np.int64)
        label = np.zeros(NPC, np.int64)
        for n in order:
            wavail = np.flatnonzero(wcount < W)
            wsel = wavail[np.argmin(wload[wavail])]
            assign[n] = wsel
            label[n] = wcount[wsel]
            wcount[wsel] += 1
            wload[wsel] += deg[n]

        ew = assign[d]
        ep = label[d]
        half = (s > LO_NODES - 1).astype(np.int64)   # s >= 32767 -> hi
        srow = np.where(half == 0, s + 1, s - LO_NODES + 1)  # idx within half

        # rank of each edge within its (w, p, half) group, sorted by srow
        okey = np.lexsort((srow, half, ep, ew))
        sw, sp, sh, sr = ew[okey], ep[okey], half[okey], srow[okey]
        gid = ((sw * W) + sp) * 2 + sh
        # rank within group
        first = np.ones(len(gid), bool)
        first[1:] = gid[1:] != gid[:-1]
        gstart = np.flatnonzero(first)
        glen = np.diff(np.append(gstart, len(gid)))
        rank = np.arange(len(gid)) - np.repeat(gstart, glen)

        perm = np.full(NPAD, -1, np.int64)
        perm[assign * W + label] = np.arange(NPC) + lo_n
        per_core_raw.append(dict(w=sw, p=sp, h=sh, r=sr, rank=rank, perm=perm))

    # ---- choose caps C_LO / C_HI minimizing total shared columns ----------
    def total_cols(t, cap):
        tot = cap * NW
        comp = np.zeros((NC, NW), np.int64)
        for c in range(NC):
            rc = per_core_raw[c]
            m = (rc["h"] == t) & (rc["rank"] >= cap)
            cnt = np.bincount(rc["w"][m], minlength=NW)
            comp[c] = (cnt + W - 1) // W
        ncomp = int(comp.max(0).sum())
        return tot + ncomp, ncomp, comp.max(0)

    # choose caps minimizing predicted critical-engine time (us):
    #   gen: ~268ns desc-gen per column (4 SWDGE queues)
    #   PE:  ~650ns per compact column (P ldweights + matmul) + fixed phases
    #   DVE: ~105ns per identity add + ~300ns per compact P-build + fixed
    lo_opts = {cap: total_cols(0, cap) for cap in range(8, 34)}
    hi_opts = {cap: total_cols(1, cap) for cap in range(4, 22)}
    best_pick = None
    for cl, (ncl, ncompl, compl) in lo_opts.items():
        for ch, (nch, ncomph, comph) in hi_opts.items():
            cols, ncomp = ncl + nch, ncompl + ncomph
            crit = max(cols * 0.268,
                       ncomp * 0.65 + 110,
                       (cl + ch) * NW * 0.105 + ncomp * 0.3 + 40)
            if best_pick is None or crit < best_pick[0]:
                best_pick = (crit, cl, ch)
    _, C_LO_pick, C_HI_pick = best_pick
    best = {0: (C_LO_pick, lo_opts[C_LO_pick][2], lo_opts[C_LO_pick][0]),
            1: (C_HI_pick, hi_opts[C_HI_pick][2], hi_opts[C_HI_pick][0])}
    C_LO, COMP_LO, n_lo = best[0]
    C_HI, COMP_HI, n_hi = best[1]
    comp_off = {0: np.concatenate([[0], np.cumsum(COMP_LO)]),
                1: np.concatenate([[0], np.cumsum(COMP_HI)])}

    # ---- global column order per half: identity cols c-major, then compact -
    # lo: cols [c * NW + w] for c<C_LO, then compact w-major; same for hi.
    n_comp_lo = int(COMP_LO.sum())
    n_comp_hi = int(COMP_HI.sum())

    per_core = []
    dep_lo = np.zeros(n_lo, np.int64)     # max row idx per lo column (shared)
    dep_hi = np.zeros(n_hi, np.int64)
    for c in range(NC):
        rc = per_core_raw[c]
        idx = {0: np.zeros(n_lo * W, np.int16), 1: np.zeros(n_hi * W, np.int16)}
        nid = np.full((n_comp_lo + n_comp_hi) * W, -1.0, np.float32)
        for t, cap, ncols_t, comp_max, coff in (
                (0, C_LO, n_lo, COMP_LO, comp_off[0]),
                (1, C_HI, n_hi, COMP_HI, comp_off[1])):
            m = rc["h"] == t
            w, p, r, rank = rc["w"][m], rc["p"][m], rc["r"][m], rc["rank"][m]
            # identity-packed part
            mi = rank < cap
            col = rank[mi] * NW + w[mi]
            slot = col * W + p[mi]
            idx[t][slot] = r[mi].astype(np.int16)
            dep_t = dep_lo if t == 0 else dep_hi
            np.maximum.at(dep_t, col, r[mi])
            # compact part
            mc = ~mi
            if mc.any():
                wc, pc, rcx = w[mc], p[mc], r[mc]
                o2 = np.lexsort((rcx, wc))
                wc, pc, rcx = wc[o2], pc[o2], rcx[o2]
                # position within window group
                fst = np.ones(len(wc), bool)
                fst[1:] = wc[1:] != wc[:-1]
                gs = np.flatnonzero(fst)
                gl = np.diff(np.append(gs, len(wc)))
                pos = np.arange(len(wc)) - np.repeat(gs, gl)
                col = cap * NW + coff[wc] + pos // W
                slot = col * W + pos % W
                idx[t][slot] = rcx.astype(np.int16)
                np.maximum.at(dep_t, col, rcx)
                comp_slot = (coff[wc] + pos // W) * W + pos % W
                nid_off = 0 if t == 0 else n_comp_lo * W
                nid[nid_off + comp_slot] = pc.astype(np.float32)

        # wrap idx into per-op [16, ni/16] layout replicated to 128 partitions
        wrapped = []
        for t, ncols_t in ((0, n_lo), (1, n_hi)):
            flat = idx[t]
            pos, blocks = 0, []
            while pos < ncols_t:
                k = min(OP_COLS, ncols_t - pos)
                op = flat[pos * W:(pos + k) * W]
                blk = op.reshape(-1, 16).T
                blocks.append(np.tile(blk, (8, 1)))
                pos += k
            wrapped.append(np.concatenate(blocks, axis=1) if blocks
                           else np.zeros((128, 8), np.int16))
        nodeid_2d = nid.reshape(n_comp_lo + n_comp_hi, W).T
        per_core.append({
            "idx_lo": np.ascontiguousarray(wrapped[0]),
            "idx_hi": np.ascontiguousarray(wrapped[1]),
            "nodeid": np.ascontiguousarray(nodeid_2d),
            "perm": rc["perm"],
        })

    # per-op dependency chunk (shared: deps are max over cores via dep_lo/hi)
    def op_deps(dep_rows, t):
        deps = []
        for o in range(0, len(dep_rows), OP_COLS):
            mx = int(dep_rows[o:o + OP_COLS].max())
            ch = mx // CHUNK + (0 if t == 0 else LO_CH)
            deps.append(ch)
        return deps

    sched = {
        "C_LO": C_LO, "C_HI": C_HI,
        "COMP_LO": COMP_LO.astype(int).tolist(),
        "COMP_HI": COMP_HI.astype(int).tolist(),
        "n_lo": int(n_lo), "n_hi": int(n_hi),
        "deps_lo": op_deps(dep_lo, 0), "deps_hi": op_deps(dep_hi, 1),
    }
    return sched, per_core


# ------------------------------------------------------------- bass program
def _build_program(sched):
    import concourse.bacc as bacc
    import concourse.mybir as mybir
    import concourse.tile as tile
    from concourse.masks import make_identity
    from concourse.tile import add_dep_helper

    f32 = mybir.dt.float32
    bf16 = mybir.dt.bfloat16
    i16 = mybir.dt.int16
    Relu = mybir.ActivationFunctionType.Relu
    Add = mybir.AluOpType.add

    C_LO, C_HI = sched["C_LO"], sched["C_HI"]
    COMP_LO, COMP_HI = sched["COMP_LO"], sched["COMP_HI"]
    n_lo, n_hi = sched["n_lo"], sched["n_hi"]
    deps = {0: sched["deps_lo"], 1: sched["deps_hi"]}
    n_comp_lo = sum(COMP_LO)
    n_comp = n_comp_lo + sum(COMP_HI)
    comp_off = {0: np.concatenate([[0], np.cumsum(COMP_LO)]),
                1: np.concatenate([[0], np.cumsum(COMP_HI)])}

    nc = bacc.Bacc(num_swdge_queues=NQ)
    yT_in = nc.dram_tensor("yT", [D + 1, MT_ROWS], bf16, kind="ExternalInput")
    wb1_in = nc.dram_tensor("wb1", [D + 1, D], bf16, kind="ExternalInput")
    wb2_in = nc.dram_tensor("wb2", [D + 1, D], bf16, kind="ExternalInput")
    ub1_in = nc.dram_tensor("ub1", [D + 1, D], f32, kind="ExternalInput")
    ub2_in = nc.dram_tensor("ub2", [D + 1, D], f32, kind="ExternalInput")
    idxlo_in = nc.dram_tensor("idx_lo", [128, n_lo * 8], i16, kind="ExternalInput")
    idxhi_in = nc.dram_tensor("idx_hi", [128, n_hi * 8], i16, kind="ExternalInput")
    nodeid_in = nc.dram_tensor("nodeid", [128, max(n_comp, 1)], f32,
                               kind="ExternalInput")
    iota_in = nc.dram_tensor("iota128", [128, 128], f32, kind="ExternalInput")
    m_dram = nc.dram_tensor("m_scratch", [MT_ROWS, D], f32, kind="Internal")
    h_out = nc.dram_tensor("h_out", [NPAD, D], f32, kind="ExternalOutput")

    with tile.TileContext(nc) as tc:
        with tc.tile_pool(name="const", bufs=1) as cpool, \
             tc.tile_pool(name="idxp", bufs=1) as idxp, \
             tc.tile_pool(name="zpool", bufs=1) as zpool:
            wb1 = cpool.tile([D + 1, D], bf16, tag="wb1")
            wb2 = cpool.tile([D + 1, D], bf16, tag="wb2")
            ub1 = cpool.tile([D + 1, D], f32, tag="ub1")
            ub2 = cpool.tile([D + 1, D], f32, tag="ub2")
            iota = cpool.tile([128, 128], f32, tag="iota")
            ident = cpool.tile([128, 128], f32, tag="ident")
            nc.sync.dma_start(out=wb1[:], in_=wb1_in[:])
            nc.sync.dma_start(out=wb2[:], in_=wb2_in[:])
            nc.sync.dma_start(out=ub1[:], in_=ub1_in[:])
            nc.sync.dma_start(out=ub2[:], in_=ub2_in[:])
            nc.sync.dma_start(out=iota[:], in_=iota_in[:])
            make_identity(nc, ident[:])

            idx_lo_t = idxp.tile([128, n_lo * 8], i16, tag="ilo")
            idx_hi_t = idxp.tile([128, n_hi * 8], i16, tag="ihi")
            idx_t = [idx_lo_t, idx_hi_t]
            nc.sync.dma_start(out=idx_lo_t[:], in_=idxlo_in[:])
            nc.sync.dma_start(out=idx_hi_t[:], in_=idxhi_in[:])
            nodeid_t = idxp.tile([128, max(n_comp, 1)], f32, tag="nid")
            nc.sync.dma_start(out=nodeid_t[:], in_=nodeid_in[:])

            # window accumulators, grouped for phase 3; zeroed up front
            NGRP = (NW + GRP - 1) // GRP
            z_grps = []
            for gi_ in range(NGRP):
                gw = min(GRP, NW - gi_ * GRP)
                zgrp = zpool.tile([128, gw * D], f32, tag=f"zg{gi_}")
                nc.vector.memset(zgrp[:], 0.0)
                z_grps.append(zgrp)

            def zslice(w):
                return z_grps[w // GRP][:, (w % GRP) * D:(w % GRP + 1) * D]

            # ------------ phase 1: m = relu(relu(y@W1+b1)@W2+b2) -> m_dram ---
            m_writes = []
            zero_writes = {}
            with tc.tile_pool(name="p1y", bufs=3) as p1y, \
                 tc.tile_pool(name="p1h", bufs=1) as p1h, \
                 tc.tile_pool(name="p1m", bufs=3) as p1m, \
                 tc.tile_pool(name="p1z", bufs=1) as p1z, \
                 tc.tile_pool(name="p1ps", bufs=2, space="PSUM") as p1ps, \
                 tc.tile_pool(name="p1ps2", bufs=3, space="PSUM") as p1ps2:
                h1a = p1h.tile([D + 1, CHUNK], bf16, tag="h1a")
                h1b = p1h.tile([D + 1, CHUNK], bf16, tag="h1b")
                nc.gpsimd.memset(h1a[D:D + 1, :], 1.0)
                nc.gpsimd.memset(h1b[D:D + 1, :], 1.0)
                h1bufs = [h1a, h1b]
                zrow = p1z.tile([1, D], f32, tag="zrow")
                nc.vector.memset(zrow[:], 0.0)
                for ch in range(NCH):
                    c0 = ch * CHUNK
                    ytile = p1y.tile([D + 1, CHUNK], bf16, tag="ytile")
                    nc.sync.dma_start(out=ytile[:], in_=yT_in[:, c0:c0 + CHUNK])
                    ps = p1ps.tile([D, CHUNK], f32, tag="ps1")
                    nc.tensor.matmul(out=ps[:], lhsT=wb1[:], rhs=ytile[:],
                                     start=True, stop=True)
                    h1c = h1bufs[ch % 2]
                    nc.scalar.activation(out=h1c[:D, :], in_=ps[:], func=Relu)
                    mch = p1m.tile([128, (CHUNK // 128) * D], f32, tag="mch")
                    for i in range(CHUNK // 128):
                        ps2 = p1ps2.tile([128, D], f32, tag="ps2")
                        nc.tensor.matmul(out=ps2[:],
                                         lhsT=h1c[:, i * 128:(i + 1) * 128],
                                         rhs=wb2[:], start=True, stop=True)
                        # ACT (not DVE): keeps the in-order DVE stream free so
                        # phase-2 adds are not head-of-line blocked by phase 1
                        nc.scalar.activation(
                            out=mch[:, i * D:(i + 1) * D], in_=ps2[:], func=Relu)
                    wri = nc.scalar.dma_start(
                        out=m_dram[c0:c0 + CHUNK, :].rearrange(
                            "(t p) d -> p t d", p=128),
                        in_=mch[:].rearrange("p (t d) -> p t d", d=D))
                    m_writes.append(wri)
                    if ch == 0 or ch == LO_CH:
                        row = 0 if ch == 0 else SPLIT
                        zw = nc.scalar.dma_start(out=m_dram[row:row + 1, :],
                                                 in_=zrow[:])
                        add_dep_helper(zw.ins, wri.ins, sync=True,
                                       reason="zero row after chunk write")
                        zero_writes[ch] = zw

            # junction chain: j[ch] fires when chunks 0..ch are all written
            junctions = []
            for ch, wri in enumerate(m_writes):
                j = nc.sync.nop(nofuse=True)
                add_dep_helper(j.ins, wri.ins, sync=True, reason="m chunk done")
                if ch in zero_writes:
                    add_dep_helper(j.ins, zero_writes[ch].ins, sync=True,
                                   reason="zero row done")
                if junctions:
                    add_dep_helper(j.ins, junctions[-1].ins, sync=True,
                                   reason="chain")
                junctions.append(j)

            # ------------ phase 2: gather + segment-sum ----------------------
            with tc.tile_pool(name="gpool", bufs=8) as gpool, \
                 tc.tile_pool(name="ppool", bufs=4) as ppool, \
                 tc.tile_pool(name="wps", bufs=4, space="PSUM") as wps:
                tables = [m_dram[0:SPLIT, :], m_dram[SPLIT:MT_ROWS, :]]
                ncols_t = [n_lo, n_hi]
                g_tiles = [{}, {}]
                op_counter = [0]

                def ensure_op(t, col):
                    o = col // OP_COLS
                    if o in g_tiles[t]:
                        return g_tiles[t][o]
                    k = min(OP_COLS, ncols_t[t] - o * OP_COLS)
                    g = gpool.tile([128, k, D], f32, tag=f"g{t}")
                    ni = k * W
                    gi = nc.gpsimd.dma_gather(
                        out_ap=g[:], in_ap=tables[t],
                        idxs_ap=idx_t[t][:, o * OP_COLS * 8:o * OP_COLS * 8 + k * 8],
                        num_idxs=ni, num_idxs_reg=ni, elem_size=D,
                        queue_num=op_counter[0] % NQ)
                    op_counter[0] += 1
                    add_dep_helper(gi.ins, junctions[deps[t][o]].ins, sync=True,
                                   reason="gather after m rows ready")
                    g_tiles[t][o] = g
                    return g

                def gcol(t, col):
                    g = ensure_op(t, col)
                    return g[:, col - (col // OP_COLS) * OP_COLS, :]

                # identity columns: plain DVE adds
                for t, cap in ((0, C_LO), (1, C_HI)):
                    for c in range(cap):
                        for w in range(NW):
                            zs = zslice(w)
                            nc.vector.tensor_tensor(
                                out=zs, in0=zs, in1=gcol(t, c * NW + w), op=Add)

                # compact columns: selection-matrix matmuls into PSUM
                for t, cap, comp, coff in ((0, C_LO, COMP_LO, comp_off[0]),
                                           (1, C_HI, COMP_HI, comp_off[1])):
                    nbase = 0 if t == 0 else n_comp_lo
                    for w in range(NW):
                        cnt = comp[w]
                        if cnt == 0:
                            continue
                        zw = wps.tile([128, D], f32, tag="zw")
                        for j in range(cnt):
                            col = cap * NW + int(coff[w]) + j
                            ncol = nbase + int(coff[w]) + j
                            P = ppool.tile([128, 128], f32, tag="P")
                            nc.vector.tensor_tensor(
                                out=P[:],
                                in0=nodeid_t[:, ncol:ncol + 1].to_broadcast(
                                    [128, 128]),
                                in1=iota[:], op=mybir.AluOpType.is_equal)
                            nc.tensor.matmul(out=zw[:], lhsT=P[:],
                                             rhs=gcol(t, col),
                                             start=(j == 0),
                                             stop=(j == cnt - 1))
                        zs = zslice(w)
                        nc.vector.tensor_tensor(out=zs, in0=zs, in1=zw[:], op=Add)

                # ------------ phase 3: h = relu(relu(z@U1+c1)@U2+c2) ---------
                with tc.tile_pool(name="p3z", bufs=3) as p3z, \
                     tc.tile_pool(name="p3g", bufs=3) as p3g, \
                     tc.tile_pool(name="p3h", bufs=3) as p3h, \
                     tc.tile_pool(name="p3ps", bufs=1, space="PSUM") as p3ps, \
                     tc.tile_pool(name="p3psb", bufs=2, space="PSUM") as p3psb, \
                     tc.tile_pool(name="p3ps2", bufs=1, space="PSUM") as p3ps2:
                    for gi_ in range(NGRP):
                        zgrp = z_grps[gi_]
                        gw = zgrp.shape[1] // D
                        cw = gw * 128
                        zTg = p3z.tile([D + 1, GRP * 128], f32, tag="zTg")
                        nc.vector.memset(zTg[D:D + 1, :cw], 1.0)
                        for k in range(gw):
                            pst = p3ps.tile([D, 128], f32, tag="pst")
                            nc.tensor.transpose(out=pst[:],
                                                in_=zgrp[:, k * D:(k + 1) * D],
                                                identity=ident[:])
                            nc.vector.tensor_copy(
                                out=zTg[:D, k * 128:(k + 1) * 128], in_=pst[:])
                        ps = p3psb.tile([D, GRP * 128], f32, tag="ps3")
                        nc.tensor.matmul(out=ps[:, :cw], lhsT=ub1[:],
                                         rhs=zTg[:, :cw], start=True, stop=True)
                        g1Tg = p3g.tile([D + 1, GRP * 128], f32, tag="g1Tg")
                        nc.scalar.activation(out=g1Tg[:D, :cw], in_=ps[:, :cw],
                                             func=Relu)
                        nc.vector.memset(g1Tg[D:D + 1, :cw], 1.0)
                        h_sb = p3h.tile([128, GRP * D], f32, tag="h_sb")
                        for k in range(gw):
                            ps2 = p3ps2.tile([128, D], f32, tag="ps4")
                            nc.tensor.matmul(out=ps2[:],
                                             lhsT=g1Tg[:, k * 128:(k + 1) * 128],
                                             rhs=ub2[:], start=True, stop=True)
                            nc.scalar.activation(
                                out=h_sb[:, k * D:(k + 1) * D], in_=ps2[:],
                                func=Relu)
                        nc.sync.dma_start(
                            out=h_out[gi_ * GRP * 128:(gi_ * GRP + gw) * 128, :]
                                .rearrange("(t p) d -> p t d", p=128),
                            in_=h_sb[:, :gw * D].rearrange("p (t d) -> p t d", d=D))

    nc.compile()
    return nc


# ------------------------------------------------------------------- kernel
def kernel(**inputs):
    from concourse.bass_utils import run_bass_kernel_spmd
    from ml_dtypes import bfloat16

    y = np.asarray(inputs["y"], np.float32)
    src = np.asarray(inputs["src"])
    dst = np.asarray(inputs["dst"])
    Ws = {k: np.asarray(inputs[k], np.float32)
          for k in ("W1", "b1", "W2", "b2", "U1", "c1", "U2", "c2")}

    sched, per_core = _host_schedule(src, dst)
    key = (sched["C_LO"], sched["C_HI"], tuple(sched["COMP_LO"]),
           tuple(sched["COMP_HI"]), tuple(sched["deps_lo"]),
           tuple(sched["deps_hi"]))
    if key not in _COMPILED:
        _COMPILED[key] = _build_program(sched)
    nc = _COMPILED[key]

    # yT in m-table row order: row 0 zero, 1..32767 = nodes 0..32766,
    # row 32768 zero, 32769.. = nodes 32767..49999
    yT = np.zeros((D + 1, MT_ROWS), np.float32)
    yT[:D, 1:LO_NODES + 1] = y[:LO_NODES].T
    yT[:D, SPLIT + 1:SPLIT + 1 + (N_NODES - LO_NODES)] = y[LO_NODES:].T
    yT[D, :] = 1.0
    yT = yT.astype(bfloat16)
    wb1 = np.concatenate([Ws["W1"], Ws["b1"][None, :]], axis=0).astype(bfloat16)
    wb2 = np.concatenate([Ws["W2"], Ws["b2"][None, :]], axis=0).astype(bfloat16)
    ub1 = np.concatenate([Ws["U1"], Ws["c1"][None, :]], axis=0)
    ub2 = np.concatenate([Ws["U2"], Ws["c2"][None, :]], axis=0)
    iota = np.tile(np.arange(128, dtype=np.float32), (128, 1))

    in_maps = []
    for c in range(NC):
        pc = per_core[c]
        nid = pc["nodeid"]
        if nid.shape[1] == 0:
            nid = np.full((128, 1), -1.0, np.float32)
        in_maps.append({
            "yT": yT, "wb1": wb1, "wb2": wb2, "ub1": ub1, "ub2": ub2,
            "idx_lo": pc["idx_lo"], "idx_hi": pc["idx_hi"],
            "nodeid": nid, "iota128": iota,
        })

    res = run_bass_kernel_spmd(nc, in_maps, core_ids=list(range(NC)),
                               trace=bool(int(os.environ.get("KERNEL_TRACE", "0"))))
    kernel.last_results = res
    kernel.last_exec_time_ns = res.exec_time_ns

    h_full = np.zeros((N_NODES, D), np.float32)
    for c in range(NC):
        out = res.results[c]["h_out"]
        perm = per_core[c]["perm"]
        valid = perm >= 0
        h_full[perm[valid]] = out[valid]
    return h_full


# revision 18
# speedup vs baseline: 2.5804x; 1.4462x over previous
"""GNN message-passing aggregator on 8 Trainium2 NeuronCores.

  h = relu(relu(z @ U1 + c1) @ U2 + c2)
  z = segment_sum(relu(relu(y[src] @ W1 + b1) @ W2 + b2), dst)

Strategy (v4):
  * MLP(y[src]) == MLP(y)[src]: compute the pre-MLP once per node (m table in
    HBM, bf16 phase 1 at 4x PE rate), then per-edge work collapses to
    gather m[src] + segment-sum by dst.
  * Edges sharded by dst ownership (6250 nodes/core): core-local segment-sum.
  * The gather is descriptor-generation bound on GPSIMD: ~8.5us per maximal
    1024-index dma_gather op, ~1.4us idle-decode on each non-owning Q7 pair.
    Ops round-robin 4 SWDGE queues -> ~3.1 ns/idx stream rate; everything
    else hides under it.
  * Identity-packed columns, banded by m-table row range: for (band b,
    level c, window w), slot p holds the c-th edge of label p whose src row
    falls in band b.  Slot == target partition, so the segment-sum is a DVE
    reduce over the band block + one add -- no PE work.  Holes point at the
    zero row at the start of each table half.
  * Banding makes gather dependencies deterministic: a band's columns only
    read rows below the band's top chunk, so its gathers start as soon as
    phase 1 has written that prefix (the m writes share one FIFO HWDGE ring,
    so one dep per op suffices).  The stream starts ~21 chunks into phase 1
    and phase 1 finishes entirely under it.
  * Edges past a band's per-label cap pool into per-(window, half) compact
    columns handled by selection-matrix matmuls into PSUM.  Compacts + the
    last hi band run window-group-major so each phase-3 group's update MLP
    pipelines into the remaining gather stream.
"""

import os

import numpy as np

# ---------------------------------------------------------------- constants
N_NODES = 50000
D = 64
NC = 8
W = 128
NPC = N_NODES // NC         # 6250
NW = (NPC + W - 1) // W     # 49
NPAD = NW * W
SPLIT = 32768
LO_NODES = SPLIT - 1        # node ids 0..32766 -> lo half
MT_ROWS = 50176             # 98 chunks of 512
CHUNK = 512
NCH = MT_ROWS // CHUNK      # 98
LO_CH = SPLIT // CHUNK      # 64
GRP = 4
NGRP = (NW + GRP - 1) // GRP
NQ = 4
OPC = 8                     # max gather op columns (1024-idx ucode limit)

# row bands per half: (start_chunk, end_chunk) within the half's row space
BANDS = {0: [(0, 21), (21, 42), (42, 64)], 1: [(0, 17), (17, 34)]}

_COMPILED = {}


# ------------------------------------------------------------ host schedule
def _host_schedule(src, dst):
    """Row map: row 0 = zero (lo), rows 1..32767 = nodes 0..32766,
    row 32768 = zero (hi), rows 32769.. = nodes 32767..49999."""
    src = np.asarray(src, np.int64)
    dst = np.asarray(dst, np.int64)

    nbands = {t: len(BANDS[t]) for t in (0, 1)}
    band_top = {t: [e * CHUNK for (_s, e) in BANDS[t]] for t in (0, 1)}

    per_core_raw = []
    for c in range(NC):
        lo_n, hi_n = c * NPC, (c + 1) * NPC
        sel = (dst >= lo_n) & (dst < hi_n)
        s = src[sel]
        d = dst[sel] - lo_n
        deg = np.bincount(d, minlength=NPC)

        order = np.argsort(-deg, kind="stable")
        wload = np.zeros(NW, np.int64)
        wcount = np.zeros(NW, np.int64)
        assign = np.zeros(NPC, np.int64)
        label = np.zeros(NPC, np.int64)
        for n in order:
            wavail = np.flatnonzero(wcount < W)
            wsel = wavail[np.argmin(wload[wavail])]
            assign[n] = wsel
            label[n] = wcount[wsel]
            wcount[wsel] += 1
            wload[wsel] += deg[n]

        ew = assign[d]
        ep = label[d]
        half = (s >= LO_NODES).astype(np.int64)
        srow = np.where(half == 0, s + 1, s - LO_NODES + 1)
        band = np.zeros(len(s), np.int64)
        for t in (0, 1):
            m = half == t
            band[m] = np.searchsorted(np.array(band_top[t]), srow[m],
                                      side="left")

        # rank within (w, p, half, band) by srow
        okey = np.lexsort((srow, band, half, ep, ew))
        sw, sp, sh, sb, sr = (ew[okey], ep[okey], half[okey], band[okey],
                              srow[okey])
        gid = (((sw * W) + sp) * 2 + sh) * 4 + sb
        first = np.ones(len(gid), bool)
        first[1:] = gid[1:] != gid[:-1]
        gstart = np.flatnonzero(first)
        glen = np.diff(np.append(gstart, len(gid)))
        rank = np.arange(len(gid)) - np.repeat(gstart, glen)

        perm = np.full(NPAD, -1, np.int64)
        perm[assign * W + label] = np.arange(NPC) + lo_n
        per_core_raw.append(dict(w=sw, p=sp, h=sh, b=sb, r=sr, rank=rank,
                                 perm=perm))

    # ---- band caps: scan per half ------------------------------------------
    def comp_counts(t, cap):
        comp = np.zeros((NC, NW), np.int64)
        for c in range(NC):
            rc = per_core_raw[c]
            m = (rc["h"] == t) & (rc["rank"] >= cap)
            cnt = np.bincount(rc["w"][m], minlength=NW)
            comp[c] = (cnt + W - 1) // W
        return comp.max(0)

    lo_opts = {cap: comp_counts(0, cap) for cap in range(3, 10)}
    hi_opts = {cap: comp_counts(1, cap) for cap in range(2, 9)}
    best_pick = None
    for cl, compl in lo_opts.items():
        for ch, comph in hi_opts.items():
            ncomp = int(compl.sum()) + int(comph.sum())
            cols = (cl * nbands[0] + ch * nbands[1]) * NW + ncomp
            # desc-gen (~0.39us/col) dominates; compacts cost a little extra
            # DVE/PE but run in bf16 so they are nearly free
            crit = cols + 0.5 * ncomp
            if best_pick is None or crit < best_pick[0]:
                best_pick = (crit, cl, ch)
    _, C_LO, C_HI = best_pick
    CAPS = {0: C_LO, 1: C_HI}
    COMP = {0: lo_opts[C_LO].astype(int), 1: hi_opts[C_HI].astype(int)}

    # ---- column + op layout ------------------------------------------------
    # sections: list of dicts(name, t, dep_chunk, cols=[(w, kind, aux)...],
    #                         group=None or group id)
    sections = []
    for t, b in [(0, 0), (0, 1), (0, 2), (1, 0)]:
        colspec = [(w, "i", (b, lev)) for w in range(NW)
                   for lev in range(CAPS[t])]
        dep = BANDS[t][b][1] - 1 + (0 if t == 0 else LO_CH)
        sections.append(dict(name=f"band{t}{b}", t=t, dep=dep, cols=colspec,
                             group=None))
    for g in range(NGRP):
        gws = list(range(g * GRP, min((g + 1) * GRP, NW)))
        lo_cols = [(w, "c", j) for w in gws for j in range(int(COMP[0][w]))]
        sections.append(dict(name=f"comp0g{g}", t=0, dep=LO_CH - 1,
                             cols=lo_cols, group=None))
        hb = nbands[1] - 1
        hi_cols = [(w, "i", (hb, lev)) for w in gws for lev in range(CAPS[1])]
        hi_cols += [(w, "c", j) for w in gws for j in range(int(COMP[1][w]))]
        sections.append(dict(name=f"hig{g}", t=1, dep=NCH - 1, cols=hi_cols,
                             group=g))

    # assign global column ids per half, ops of <= OPC columns per section
    col_index = {}
    n_cols = {0: 0, 1: 0}
    ops = {0: [], 1: []}        # (start, k, dep)
    for sec in sections:
        t = sec["t"]
        sec["col0"] = n_cols[t]
        for (w, kind, aux) in sec["cols"]:
            col_index[(t, w, kind, aux)] = n_cols[t]
            n_cols[t] += 1
        start = sec["col0"]
        while start < n_cols[t]:
            k = min(OPC, n_cols[t] - start)
            sec.setdefault("ops", []).append(len(ops[t]))
            ops[t].append((start, k, sec["dep"]))
            start += k

    comp_col_id = {}
    nid_cols = 0
    for (t, w, kind, aux), ci in sorted(col_index.items(),
                                        key=lambda kv: (kv[0][0], kv[1])):
        if kind == "c":
            comp_col_id[(t, ci)] = nid_cols
            nid_cols += 1

    # ---- per-core planes ---------------------------------------------------
    per_core = []
    for c in range(NC):
        rc = per_core_raw[c]
        idx = {0: np.zeros(n_cols[0] * W, np.int16),
               1: np.zeros(n_cols[1] * W, np.int16)}
        nid = np.full(max(nid_cols, 1) * W, -1.0, np.float32)
        for t in (0, 1):
            cap = CAPS[t]
            m = rc["h"] == t
            w, p, b, r, rank = (rc["w"][m], rc["p"][m], rc["b"][m],
                                rc["r"][m], rc["rank"][m])
            mi = rank < cap
            colv = np.array([col_index[(t, int(a), "i", (int(bb), int(k)))]
                             for a, bb, k in zip(w[mi], b[mi], rank[mi])],
                            np.int64)
            idx[t][colv * W + p[mi]] = r[mi].astype(np.int16)
            mc = ~mi
            if mc.any():
                wc, pc, rcx = w[mc], p[mc], r[mc]
                o2 = np.lexsort((rcx, wc))
                wc, pc, rcx = wc[o2], pc[o2], rcx[o2]
                fst = np.ones(len(wc), bool)
                fst[1:] = wc[1:] != wc[:-1]
                gs = np.flatnonzero(fst)
                gl = np.diff(np.append(gs, len(wc)))
                pos = np.arange(len(wc)) - np.repeat(gs, gl)
                colv = np.array([col_index[(t, int(a), "c", int(j))]
                                 for a, j in zip(wc, pos // W)], np.int64)
                slot = colv * W + pos % W
                idx[t][slot] = rcx.astype(np.int16)
                nidcol = np.array([comp_col_id[(t, int(cv))] for cv in colv],
                                  np.int64)
                nid[nidcol * W + pos % W] = pc.astype(np.float32)

        wrapped = {}
        for t in (0, 1):
            blocks = []
            for (start, k, _dep) in ops[t]:
                op = idx[t][start * W:(start + k) * W]
                blocks.append(np.tile(op.reshape(-1, 16).T, (8, 1)))
            wrapped[t] = np.ascontiguousarray(np.concatenate(blocks, axis=1))
        per_core.append({
            "idx_lo": wrapped[0], "idx_hi": wrapped[1],
            "nodeid": np.ascontiguousarray(
                nid.reshape(max(nid_cols, 1), W).T),
            "perm": rc["perm"],
        })

    # idx byte offset of each op in the wrapped planes
    op_off = {t: np.cumsum([0] + [k * 8 for (_s, k, _d) in ops[t]]).tolist()
              for t in (0, 1)}

    sched = {
        "CAPS": CAPS, "COMP": {t: COMP[t].tolist() for t in (0, 1)},
        "sections": sections, "ops": ops, "op_off": op_off,
        "n_cols": n_cols, "nid_cols": nid_cols, "comp_col_id": comp_col_id,
        "col_index": col_index,
    }
    return sched, per_core


def _sched_key(sched):
    return (tuple(sorted(sched["CAPS"].items())),
            tuple(tuple(sched["COMP"][t]) for t in (0, 1)))


# ------------------------------------------------------------- bass program
def _build_program(sched):
    import concourse.bacc as bacc
    import concourse.mybir as mybir
    import concourse.tile as tile
    from concourse.masks import make_identity
    from concourse.tile import add_dep_helper

    f32 = mybir.dt.float32
    bf16 = mybir.dt.bfloat16
    i16 = mybir.dt.int16
    Relu = mybir.ActivationFunctionType.Relu
    Add = mybir.AluOpType.add
    X = mybir.AxisListType.X

    CAPS = sched["CAPS"]
    sections = sched["sections"]
    ops = sched["ops"]
    op_off = sched["op_off"]
    n_cols = sched["n_cols"]
    nid_cols = max(sched["nid_cols"], 1)
    comp_col_id = sched["comp_col_id"]

    nc = bacc.Bacc(num_swdge_queues=NQ)
    yT_in = nc.dram_tensor("yT", [D + 1, MT_ROWS], bf16, kind="ExternalInput")
    wb1_in = nc.dram_tensor("wb1", [D + 1, D], bf16, kind="ExternalInput")
    wb2_in = nc.dram_tensor("wb2", [D + 1, D], bf16, kind="ExternalInput")
    ub1_in = nc.dram_tensor("ub1", [D + 1, D], f32, kind="ExternalInput")
    ub2_in = nc.dram_tensor("ub2", [D + 1, D], bf16, kind="ExternalInput")
    idxlo_in = nc.dram_tensor("idx_lo", [128, n_cols[0] * 8], i16,
                              kind="ExternalInput")
    idxhi_in = nc.dram_tensor("idx_hi", [128, n_cols[1] * 8], i16,
                              kind="ExternalInput")
    nodeid_in = nc.dram_tensor("nodeid", [128, nid_cols], f32,
                               kind="ExternalInput")
    iota_in = nc.dram_tensor("iota128", [128, 128], f32, kind="ExternalInput")
    m_dram = nc.dram_tensor("m_scratch", [MT_ROWS, D], f32, kind="Internal")
    h_out = nc.dram_tensor("h_out", [NPAD, D], f32, kind="ExternalOutput")

    with tile.TileContext(nc) as tc:
        with tc.tile_pool(name="const", bufs=1) as cpool, \
             tc.tile_pool(name="idxp", bufs=1) as idxp, \
             tc.tile_pool(name="zpool", bufs=1) as zpool:
            wb1 = cpool.tile([D + 1, D], bf16, tag="wb1")
            wb2 = cpool.tile([D + 1, D], bf16, tag="wb2")
            ub1 = cpool.tile([D + 1, D], f32, tag="ub1")
            ub2 = cpool.tile([D + 1, D], bf16, tag="ub2")
            iota = cpool.tile([128, 128], f32, tag="iota")
            ident = cpool.tile([128, 128], f32, tag="ident")
            nc.sync.dma_start(out=wb1[:], in_=wb1_in[:])
            nc.sync.dma_start(out=wb2[:], in_=wb2_in[:])
            nc.sync.dma_start(out=ub1[:], in_=ub1_in[:])
            nc.sync.dma_start(out=ub2[:], in_=ub2_in[:])
            nc.sync.dma_start(out=iota[:], in_=iota_in[:])
            make_identity(nc, ident[:])

            idx_lo_t = idxp.tile([128, n_cols[0] * 8], i16, tag="ilo")
            idx_hi_t = idxp.tile([128, n_cols[1] * 8], i16, tag="ihi")
            idx_t = [idx_lo_t, idx_hi_t]
            nc.sync.dma_start(out=idx_lo_t[:], in_=idxlo_in[:])
            nc.sync.dma_start(out=idx_hi_t[:], in_=idxhi_in[:])
            nodeid_t = idxp.tile([128, nid_cols], f32, tag="nid")
            nc.sync.dma_start(out=nodeid_t[:], in_=nodeid_in[:])

            z_grps = []
            for gi_ in range(NGRP):
                gw = min(GRP, NW - gi_ * GRP)
                zgrp_t = zpool.tile([128, gw * D], f32, tag=f"zg{gi_}")
                z_grps.append(zgrp_t)

            def zslice(w):
                return z_grps[w // GRP][:, (w % GRP) * D:(w % GRP + 1) * D]

            # ------------ phase 1 -------------------------------------------
            m_writes = []
            zero_writes = {}
            with tc.tile_pool(name="p1y", bufs=3) as p1y, \
                 tc.tile_pool(name="p1h", bufs=1) as p1h, \
                 tc.tile_pool(name="p1m", bufs=3) as p1m, \
                 tc.tile_pool(name="p1z", bufs=1) as p1z, \
                 tc.tile_pool(name="p1ps", bufs=2, space="PSUM") as p1ps, \
                 tc.tile_pool(name="p1ps2", bufs=3, space="PSUM") as p1ps2:
                h1a = p1h.tile([D + 1, CHUNK], bf16, tag="h1a")
                h1b = p1h.tile([D + 1, CHUNK], bf16, tag="h1b")
                nc.gpsimd.memset(h1a[D:D + 1, :], 1.0)
                nc.gpsimd.memset(h1b[D:D + 1, :], 1.0)
                h1bufs = [h1a, h1b]
                zrow = p1z.tile([1, D], f32, tag="zrow")
                nc.vector.memset(zrow[:], 0.0)
                for ch in range(NCH):
                    c0 = ch * CHUNK
                    ytile = p1y.tile([D + 1, CHUNK], bf16, tag="ytile")
                    nc.sync.dma_start(out=ytile[:], in_=yT_in[:, c0:c0 + CHUNK])
                    ps = p1ps.tile([D, CHUNK], f32, tag="ps1")
                    nc.tensor.matmul(out=ps[:], lhsT=wb1[:], rhs=ytile[:],
                                     start=True, stop=True)
                    h1c = h1bufs[ch % 2]
                    nc.scalar.activation(out=h1c[:D, :], in_=ps[:], func=Relu)
                    ps2 = p1ps2.tile([128, (CHUNK // 128) * D], f32, tag="ps2")
                    for i in range(CHUNK // 128):
                        nc.tensor.matmul(out=ps2[:, i * D:(i + 1) * D],
                                         lhsT=h1c[:, i * 128:(i + 1) * 128],
                                         rhs=wb2[:], start=True, stop=True)
                    mch = p1m.tile([128, (CHUNK // 128) * D], f32, tag="mch")
                    nc.scalar.activation(out=mch[:], in_=ps2[:], func=Relu)
                    wri = nc.scalar.dma_start(
                        out=m_dram[c0:c0 + CHUNK, :].rearrange(
                            "(t p) d -> p t d", p=128),
                        in_=mch[:].rearrange("p (t d) -> p t d", d=D))
                    m_writes.append(wri)
                    if ch == 0 or ch == LO_CH:
                        row = 0 if ch == 0 else SPLIT
                        zw = nc.scalar.dma_start(out=m_dram[row:row + 1, :],
                                                 in_=zrow[:])
                        zero_writes[ch] = zw

            def dep_ins(chv):
                if chv in zero_writes:
                    return zero_writes[chv]
                return m_writes[chv]

            # ------------ phase 2+3 -----------------------------------------
            with tc.tile_pool(name="gpool", bufs=8) as gpool, \
                 tc.tile_pool(name="tpool", bufs=3) as tpool, \
                 tc.tile_pool(name="ppool", bufs=4) as ppool, \
                 tc.tile_pool(name="gbp", bufs=4) as gbpool, \
                 tc.tile_pool(name="wps", bufs=4, space="PSUM") as wps, \
                 tc.tile_pool(name="p3z", bufs=3) as p3z, \
                 tc.tile_pool(name="p3g", bufs=3) as p3g, \
                 tc.tile_pool(name="p3h", bufs=3) as p3h, \
                 tc.tile_pool(name="p3ps", bufs=1, space="PSUM") as p3ps, \
                 tc.tile_pool(name="p3psb", bufs=2, space="PSUM") as p3psb, \
                 tc.tile_pool(name="p3ps2", bufs=1, space="PSUM") as p3ps2:
                tables = [m_dram[0:SPLIT, :], m_dram[SPLIT:MT_ROWS, :]]
                g_tiles = {}
                op_counter = [0]

                def run_op(t, oi):
                    if (t, oi) in g_tiles:
                        return g_tiles[(t, oi)]
                    start, k, dep = ops[t][oi]
                    g = gpool.tile([128, k, D], f32, tag="g")
                    ni = k * W
                    gi = nc.gpsimd.dma_gather(
                        out_ap=g[:], in_ap=tables[t],
                        idxs_ap=idx_t[t][:, op_off[t][oi]:
                                         op_off[t][oi] + k * 8],
                        num_idxs=ni, num_idxs_reg=ni, elem_size=D,
                        queue_num=op_counter[0] % NQ)
                    op_counter[0] += 1
                    add_dep_helper(gi.ins, dep_ins(dep).ins, sync=True,
                                   reason="m rows ready")
                    g_tiles[(t, oi)] = g
                    return g

                zinit = set()

                def consume_ident_run(t, ci0, w, count):
                    """Reduce `count` identity cols (global col ci0..) of
                    window w into zgrp, spanning op boundaries."""
                    zs = zslice(w)
                    ci = ci0
                    left = count
                    while left > 0:
                        oi = next(i for i, (s, k, _d) in enumerate(ops[t])
                                  if s <= ci < s + k)
                        s, k, _d = ops[t][oi]
                        g = run_op(t, oi)
                        off = ci - s
                        n = min(left, k - off)
                        view = g[:, off:off + n, :].rearrange("p j d -> p d j")
                        if w not in zinit:
                            nc.vector.tensor_reduce(out=zs, in_=view,
                                                    axis=X, op=Add)
                            zinit.add(w)
                        elif n == 1:
                            nc.vector.tensor_tensor(out=zs, in0=zs,
                                                    in1=g[:, off, :], op=Add)
                        else:
                            tmp = tpool.tile([128, D], f32, tag="tmp")
                            nc.vector.tensor_reduce(out=tmp[:], in_=view,
                                                    axis=X, op=Add)
                            nc.vector.tensor_tensor(out=zs, in0=zs,
                                                    in1=tmp[:], op=Add)
                        ci += n
                        left -= n

                def consume_comp_run(t, ci0, w, count):
                    if count == 0:
                        return
                    zw = wps.tile([128, D], f32, tag="zw")
                    for j in range(count):
                        ci = ci0 + j
                        oi = next(i for i, (s, k, _d) in enumerate(ops[t])
                                  if s <= ci < s + k)
                        s, k, _d = ops[t][oi]
                        g = run_op(t, oi)
                        ncol = comp_col_id[(t, ci)]
                        gb = gbpool.tile([128, D], bf16, tag="gb")
                        nc.scalar.copy(out=gb[:], in_=g[:, ci - s, :])
                        P = ppool.tile([128, 128], bf16, tag="P")
                        nc.vector.tensor_tensor(
                            out=P[:],
                            in0=nodeid_t[:, ncol:ncol + 1].to_broadcast(
                                [128, 128]),
                            in1=iota[:], op=mybir.AluOpType.is_equal)
                        nc.tensor.matmul(out=zw[:], lhsT=P[:], rhs=gb[:],
                                         start=(j == 0), stop=(j == count - 1))
                    zs = zslice(w)
                    nc.vector.tensor_tensor(out=zs, in0=zs, in1=zw[:], op=Add)

                col_index = sched["col_index"]

                def sec_consume(sec):
                    t = sec["t"]
                    # walk cols grouped by (w, kind-run)
                    i = 0
                    colspec = sec["cols"]
                    while i < len(colspec):
                        w, kind, _aux = colspec[i]
                        j = i
                        while (j < len(colspec) and colspec[j][0] == w
                               and colspec[j][1] == kind):
                            j += 1
                        ci0 = col_index[(t,) + tuple(colspec[i][:1]) +
                                        (kind, colspec[i][2])] \
                            if False else \
                            col_index[(t, w, kind, colspec[i][2])]
                        if kind == "i":
                            consume_ident_run(t, ci0, w, j - i)
                        else:
                            consume_comp_run(t, ci0, w, j - i)
                        i = j

                def phase3(gi_):
                    zgrp = z_grps[gi_]
                    gw = zgrp.shape[1] // D
                    cw = gw * 128
                    zTg = p3z.tile([D + 1, GRP * 128], f32, tag="zTg")
                    nc.vector.memset(zTg[D:D + 1, :cw], 1.0)
                    for k in range(gw):
                        pst = p3ps.tile([D, 128], f32, tag="pst")
                        nc.tensor.transpose(out=pst[:],
                                            in_=zgrp[:, k * D:(k + 1) * D],
                                            identity=ident[:])
                        nc.vector.tensor_copy(
                            out=zTg[:D, k * 128:(k + 1) * 128], in_=pst[:])
                    ps3 = p3psb.tile([D, GRP * 128], f32, tag="ps3")
                    nc.tensor.matmul(out=ps3[:, :cw], lhsT=ub1[:],
                                     rhs=zTg[:, :cw], start=True, stop=True)
                    g1Tg = p3g.tile([D + 1, GRP * 128], bf16, tag="g1Tg")
                    nc.scalar.activation(out=g1Tg[:D, :cw], in_=ps3[:, :cw],
                                         func=Relu)
                    nc.vector.memset(g1Tg[D:D + 1, :cw], 1.0)
                    h_sb = p3h.tile([128, GRP * D], f32, tag="h_sb")
                    for k in range(gw):
                        ps4 = p3ps2.tile([128, D], f32, tag="ps4")
                        nc.tensor.matmul(out=ps4[:],
                                         lhsT=g1Tg[:, k * 128:(k + 1) * 128],
                                         rhs=ub2[:], start=True, stop=True)
                        nc.scalar.activation(
                            out=h_sb[:, k * D:(k + 1) * D], in_=ps4[:],
                            func=Relu)
                    nc.sync.dma_start(
                        out=h_out[gi_ * GRP * 128:(gi_ * GRP + gw) * 128, :]
                            .rearrange("(t p) d -> p t d", p=128),
                        in_=h_sb[:, :gw * D].rearrange("p (t d) -> p t d", d=D))

                for sec in sections:
                    sec_consume(sec)
                    if sec["group"] is not None:
                        phase3(sec["group"])

    nc.compile()
    return nc


# ------------------------------------------------------------------- kernel
def kernel(**inputs):
    from concourse.bass_utils import run_bass_kernel_spmd
    from ml_dtypes import bfloat16

    y = np.asarray(inputs["y"], np.float32)
    src = np.asarray(inputs["src"])
    dst = np.asarray(inputs["dst"])
    Ws = {k: np.asarray(inputs[k], np.float32)
          for k in ("W1", "b1", "W2", "b2", "U1", "c1", "U2", "c2")}

    sched, per_core = _host_schedule(src, dst)
    key = _sched_key(sched)
    if key not in _COMPILED:
        _COMPILED[key] = _build_program(sched)
    nc = _COMPILED[key]

    yT = np.zeros((D + 1, MT_ROWS), np.float32)
    yT[:D, 1:LO_NODES + 1] = y[:LO_NODES].T
    yT[:D, SPLIT + 1:SPLIT + 1 + (N_NODES - LO_NODES)] = y[LO_NODES:].T
    yT[D, :] = 1.0
    yT = yT.astype(bfloat16)
    wb1 = np.concatenate([Ws["W1"], Ws["b1"][None, :]], axis=0).astype(bfloat16)
    wb2 = np.concatenate([Ws["W2"], Ws["b2"][None, :]], axis=0).astype(bfloat16)
    ub1 = np.concatenate([Ws["U1"], Ws["c1"][None, :]], axis=0)
    ub2 = np.concatenate([Ws["U2"], Ws["c2"][None, :]],
                         axis=0).astype(bfloat16)
    iota = np.tile(np.arange(128, dtype=np.float32), (128, 1))

    in_maps = []
    for c in range(NC):
        pc = per_core[c]
        in_maps.append({
            "yT": yT, "wb1": wb1, "wb2": wb2, "ub1": ub1, "ub2": ub2,
            "idx_lo": pc["idx_lo"], "idx_hi": pc["idx_hi"],
            "nodeid": pc["nodeid"], "iota128": iota,
        })

    res = run_bass_kernel_spmd(nc, in_maps, core_ids=list(range(NC)),
                               trace=bool(int(os.environ.get("KERNEL_TRACE", "0"))))
    kernel.last_results = res
    kernel.last_exec_time_ns = res.exec_time_ns

    h_full = np.zeros((N_NODES, D), np.float32)
    for c in range(NC):
        out = res.results[c]["h_out"]
        perm = per_core[c]["perm"]
        valid = perm >= 0
        h_full[perm[valid]] = out[valid]
    return h_full
